# revision 9
# baseline (speedup 1.0000x reference)
"""Trainium2 Bass kernel for nn_Embedded_GCN (gnn_message_passing).

Reference math (B=32, N=4096, C=32, O=64, D=16, K=3):
  A  = softmax(relu(E @ E.T), axis=1)              # [N, N] adaptive adjacency
  T0 = I, T1 = A, T2 = 2A@A - I                    # Chebyshev
  x_g[k]   = T_k @ x_b  for each batch             # [B, K, N, C]
  W[n]     = sum_d E[n,d] * Wp[d]                  # per-node weights [K,C,O]
  out[b,n] = sum_{k,i} x_g[b,n,k,i] W[n,k,i,:] + E[n]@bias_pool

Schedule (v4): hops split into bc-column halves; scores+exp software-pipelined
into hop1 sweep A (PE never idles behind the Act exp chain); xt's second
column half is cached in SBUF during the fused phase so sweep B needs no DMA
while AllGather #1 is in flight; a tiny warmup AllGather at program start
absorbs the inter-core skew so AG1's mesh handshake is short; hop2 consumes
ag_out0 for h=0 right after weight-gen and ag_out1 for h=1. Grouped-GEMM
output is dumped in raw PSUM-tile layout (contiguous) and reassembled on the
host.

  * z1 = A@x, z2 = 2*A@z1 - x (T2 never materialized).
  * softmax(relu(s)) = max(1, exp(s)) / rowsum.
  * PT[m, n] = transposed exp-scores (contraction m on partitions).
  * Scores f32r; both hops bf16.
  * Row sums: DVE/Pool split interleaved accumulation of exp tiles, then 4
    tiny PE ones-matmuls; 1/Z folded into the hop epilogues.
  * Per-node grouped GEMM: x_g permuted to [(k,c), (n,b)] via a DRAM
    round-trip, per-node weights generated on the PE into a
    [97, (n_hi, o, n_lo=8)]-interleaved bf16 slab (97th row = bias), and the
    512 per-node [97,32]x[97,64] bf16 matmuls run col-tiled via tile_position.
"""

import os

import numpy as np
import ml_dtypes

import concourse.bass as bass
import concourse.mybir as mybir
import concourse.tile as tile
from concourse import bacc
from concourse.bass_utils import run_bass_kernel_spmd

F32 = mybir.dt.float32
F32R = mybir.dt.float32r
BF16 = mybir.dt.bfloat16
AF = mybir.ActivationFunctionType

B, N, C, O, D, CHEB_K = 32, 4096, 32, 64, 16, 3
NC_CORES = 8
NL = N // NC_CORES          # 512 nodes per core
BC = B * C                  # 1024
MT = N // 128               # 32 contraction tiles
NT = NL // 128              # 4 local node tiles

LAST_RESULTS = {}


def _register_ntff_hook():
    """Inject antenv.axon_hooks (absent from the container's antenv stub) and
    register the ctypes NTFF-profile hook so trace=True works under axon."""
    import sys
    import types

    try:
        import antenv

        if "antenv.axon_hooks" not in sys.modules:
            mod = types.ModuleType("antenv.axon_hooks")
            mod._hook = None

            def set_axon_ntff_profile_hook(h):
                mod._hook = h

            def get_axon_ntff_profile_hook():
                return mod._hook

            mod.set_axon_ntff_profile_hook = set_axon_ntff_profile_hook
            mod.get_axon_ntff_profile_hook = get_axon_ntff_profile_hook
            sys.modules["antenv.axon_hooks"] = mod
            antenv.axon_hooks = mod

        hooks = sys.modules["antenv.axon_hooks"]
        if hooks.get_axon_ntff_profile_hook() is None:
            from trn_agent_boot.trn_boot import _ntff_profile_via_ctypes

            hook = _ntff_profile_via_ctypes("/opt/axon/libaxon_pjrt.so")
            if hook is not None:
                hooks.set_axon_ntff_profile_hook(hook)
        return True
    except Exception:
        return False


def _build(nc: bacc.Bacc):
    # ---- I/O -------------------------------------------------------------
    et = nc.dram_tensor("et", [D, N], F32, kind="ExternalInput")          # E^T
    et_loc = nc.dram_tensor("et_loc", [D, NL], F32, kind="ExternalInput")
    xt_bf = nc.dram_tensor("xt_bf", [N, BC], BF16, kind="ExternalInput")   # x[b,m,c] -> [m, c*32+b]
    xtl_bf = nc.dram_tensor("xtl_bf", [NL, BC], BF16, kind="ExternalInput")
    xt_loc = nc.dram_tensor("xt_loc", [NL, BC], F32, kind="ExternalInput")
    wp_o = nc.dram_tensor("wp_o", [O, D, CHEB_K * C], BF16, kind="ExternalInput")
    bias_flat = nc.dram_tensor("bias_flat", [1, NL * O], BF16, kind="ExternalInput")
    ones_row = nc.dram_tensor("ones_row", [1, 128 * B], BF16, kind="ExternalInput")
    # raw grouped-GEMM PSUM dump: [(ch,q16)*128 + (g,b), (j,o)]; host reassembles
    out_hw = nc.dram_tensor("out_hw", [16 * 128, 512], F32, kind="ExternalOutput")

    with tile.TileContext(nc) as tc:
        with tc.tile_pool(name="dram", bufs=1, space="DRAM") as dram, \
             tc.tile_pool(name="persist", bufs=1) as persist:

            warm_in = dram.tile([1, 128], BF16, tag="warm_in", name="warm_in")
            warm_out = dram.tile([NC_CORES, 128], BF16, tag="warm_out",
                                 name="warm_out", addr_space="Shared")
            # z1 column halves: ag_in[q] = local 512 nodes x bc half q
            ag_ins = [dram.tile([NL, 512], BF16, tag=f"ag_in{q}", name=f"ag_in{q}")
                      for q in range(2)]
            ag_outs = [dram.tile([N, 512], BF16, tag=f"ag_out{q}",
                                 name=f"ag_out{q}", addr_space="Shared")
                       for q in range(2)]
            scr1 = dram.tile([C, NL, B], BF16, tag="scr1")   # z1 as [c, n, b]
            scr2 = dram.tile([C, NL, B], BF16, tag="scr2")   # z2 as [c, n, b]

            # ---- small persistent SBUF ------------------------------------
            etl_sb = persist.tile([D, NL], F32R, tag="etl")
            r1 = persist.tile([128, NT], F32, tag="r1")          # 1/Z  per node col nt
            r2 = persist.tile([128, NT], F32, tag="r2")          # 2/Z
            ones_f = persist.tile([128, 2], F32, tag="onesf")
            etl_bf = persist.tile([D, NL], BF16, tag="etlbf")
            wpo_sb = persist.tile([D, O * CHEB_K * C], BF16, tag="wpo")
            warm_sb = persist.tile([1, 128], BF16, tag="warmsb")

            # warmup collective: sync the 8 cores so AG1's rendezvous is short
            nc.gpsimd.dma_start(warm_in[:], xt_bf[0:1, 0:128])
            nc.gpsimd.collective_compute(
                "AllGather",
                mybir.AluOpType.bypass,
                ins=[warm_in.opt()],
                outs=[warm_out.opt()],
                replica_groups=[list(range(NC_CORES))],
            )
            nc.gpsimd.dma_start(warm_sb[:], warm_out[0:1, :])

            nc.sync.dma_start(etl_sb[:], et_loc[:, :].bitcast(F32R))
            nc.vector.memset(ones_f[:], 1.0)
            nc.vector.tensor_copy(etl_bf[:], etl_sb[:].bitcast(F32))

            with tc.tile_pool(name="xgp", bufs=1) as xgp, \
                 tc.tile_pool(name="tstream", bufs=3) as tstream:
                xgs = [xgp.tile([97, 128 * B], BF16, tag=f"xg{ch}", name=f"xg{ch}")
                       for ch in range(NT)]

                with tc.tile_pool(name="ptp", bufs=1) as ptp, \
                     tc.tile_pool(name="stream", bufs=4) as stream:
                    pt = ptp.tile([128, MT * NL], BF16, tag="pt")  # PT[m%128, mt*NL+n]
                    xloc_sb = ptp.tile([128, NT * BC], F32, tag="xloc")
                    accs = [ptp.tile([128, NL], F32, tag=f"accs{i}", name=f"accs{i}")
                            for i in range(2)]

                    # ==== early epoch: fused scores+sweepA, sweepB (SBUF) ====
                    xt2p_cm = tc.tile_pool(name="xt2p", bufs=1)
                    xt2p = xt2p_cm.__enter__()
                    xt2 = xt2p.tile([128, MT * 512], BF16, tag="xt2")
                    z1sb = [None] * (2 * NT)

                    ps_acc_cm = tc.tile_pool(name="ps_acc", bufs=1, space="PSUM")
                    ps_acc = ps_acc_cm.__enter__()
                    acc = [ps_acc.tile([128, 512], F32, tag=f"acc{nt_i}",
                                       name=f"acc{nt_i}") for nt_i in range(NT)]

                    def hop1_mms(k, rhs_ap):
                        for nt_i in range(NT):
                            nc.tensor.matmul(
                                acc[nt_i][:],
                                pt[:, k * NL + nt_i * 128: k * NL + (nt_i + 1) * 128],
                                rhs_ap,
                                start=(k == 0), stop=(k == MT - 1),
                            )

                    with tc.tile_pool(name="etp", bufs=2) as etp, \
                         tc.tile_pool(name="ps_sc", bufs=2, space="PSUM") as ps_sc, \
                         tc.tile_pool(name="ps_zs", bufs=1, space="PSUM") as ps_zs:
                        et_c = None
                        rts = [None] * MT
                        for mt in range(MT):
                            if mt % 8 == 0:
                                et_c = etp.tile([D, 1024], F32R, tag="etc")
                                nc.sync.dma_start(
                                    et_c[:],
                                    et[:, mt * 128:(mt + 8) * 128].bitcast(F32R))
                            s_ps = ps_sc.tile([128, NL], F32, tag="s")
                            nc.tensor.matmul(
                                s_ps[:],
                                et_c[:, (mt % 8) * 128:(mt % 8 + 1) * 128],
                                etl_sb[:],
                                start=True, stop=True,
                            )
                            pslice = pt[:, mt * NL:(mt + 1) * NL]
                            nc.scalar.activation(pslice, s_ps[:], AF.Exp)
                            nc.vector.tensor_scalar_max(pslice, pslice, 1.0)
                            # rowsum accumulation split across DVE and Pool
                            eng = nc.vector if mt % 2 == 0 else nc.gpsimd
                            a = accs[mt % 2]
                            if mt < 2:
                                eng.tensor_copy(a[:], pslice)
                            else:
                                eng.tensor_tensor(
                                    a[:], a[:], pslice, mybir.AluOpType.add)
                            rt = stream.tile([128, 512], BF16, tag="rhs")
                            nc.sync.dma_start(
                                rt[:], xt_bf[mt * 128:(mt + 1) * 128, 0:512])
                            rts[mt] = rt
                            # xt second-half cache fill (for DMA-free sweep B)
                            nc.scalar.dma_start(
                                xt2[:, mt * 512:(mt + 1) * 512],
                                xt_bf[mt * 128:(mt + 1) * 128, 512:1024])
                            if mt == 0:
                                nc.scalar.dma_start(
                                    wpo_sb[:].rearrange(
                                        "d (o k) -> d o k", k=CHEB_K * C),
                                    wp_o[:, :, :].transpose((1, 0, 2)),
                                )
                            if mt >= 2:
                                hop1_mms(mt - 2, rts[mt - 2][:, :])
                                rts[mt - 2] = None
                        hop1_mms(MT - 2, rts[MT - 2][:, :])
                        hop1_mms(MT - 1, rts[MT - 1][:, :])

                        # row sums -> r1 = 1/Z, r2 = 2/Z
                        nc.vector.tensor_tensor(
                            accs[0][:], accs[0][:], accs[1][:], mybir.AluOpType.add)
                        zs = ps_zs.tile([128, 8], F32, tag="zs")
                        for nt_i in range(NT):
                            nc.tensor.matmul(
                                zs[:, 2 * nt_i:2 * nt_i + 2],
                                accs[0][:, nt_i * 128:(nt_i + 1) * 128],
                                ones_f[:],
                                start=True, stop=True,
                            )
                        for nt_i in range(NT):
                            nc.vector.reciprocal(
                                r1[:, nt_i:nt_i + 1], zs[:, 2 * nt_i:2 * nt_i + 1])
                            nc.vector.tensor_scalar_mul(
                                r2[:, nt_i:nt_i + 1], r1[:, nt_i:nt_i + 1], 2.0)

                    # ---- drain sweep A (h=0) -> AG1; sweep B (h=1) -> AG2 -
                    def drain(h):
                        for nt_i in range(NT):
                            st_bf = xt2p.tile([128, 512], BF16, tag=f"zb{h}_{nt_i}",
                                              name=f"zb{h}_{nt_i}")
                            nc.vector.tensor_scalar(
                                st_bf[:], acc[nt_i][:], r1[:, nt_i:nt_i + 1], None,
                                op0=mybir.AluOpType.mult,
                            )
                            z1sb[h * NT + nt_i] = st_bf
                            nc.sync.dma_start(
                                ag_ins[h][nt_i * 128:(nt_i + 1) * 128, :], st_bf[:])
                            nc.scalar.dma_start(
                                scr1[h * 16:(h + 1) * 16,
                                     nt_i * 128:(nt_i + 1) * 128, :]
                                .transpose((1, 0, 2)),
                                st_bf[:].rearrange("p (c b) -> p c b", b=B),
                            )
                        nc.gpsimd.collective_compute(
                            "AllGather",
                            mybir.AluOpType.bypass,
                            ins=[ag_ins[h].opt()],
                            outs=[ag_outs[h].opt()],
                            replica_groups=[list(range(NC_CORES))],
                        )

                    drain(0)

                    # sweep B: same PT, bc cols 512:1024 from SBUF cache
                    for k in range(MT):
                        hop1_mms(k, xt2[:, k * 512:(k + 1) * 512])
                        if k == 2:
                            for ch in range(NT):
                                n0 = ch * 128
                                nc.gpsimd.dma_start(
                                    xgs[ch][0:C, :].rearrange("c (n b) -> c n b", b=B),
                                    xtl_bf[n0:n0 + 128, :].rearrange(
                                        "n (c b) -> c n b", b=B),
                                )
                        if k == 8:
                            for ch in range(NT):
                                nc.gpsimd.dma_start(xgs[ch][96:97, :], ones_row[:, :])
                        if k == 12:
                            # z1 h=0 rows of the xg slabs (scr1 h0 written above)
                            for ch in range(NT):
                                nc.gpsimd.dma_start(
                                    xgs[ch][C:C + 16, :].rearrange(
                                        "c (n b) -> c n b", b=B),
                                    scr1[0:16, ch * 128:(ch + 1) * 128, :],
                                )
                        if k == 16:
                            nc.scalar.dma_start(
                                xloc_sb[:].rearrange("p (t f) -> p t f", f=BC),
                                xt_loc[:, :].rearrange("(t p) f -> p t f", p=128),
                            )

                    drain(1)
                    ps_acc_cm.__exit__(None, None, None)
                    xt2p_cm.__exit__(None, None, None)

                    # ==== late epoch: weight slab + hop2 + grouped ====
                    with tc.tile_pool(name="wtp", bufs=1) as wtp:
                        # weight slab, n-major: [ki|bias, (n, o)]
                        wt_bf = wtp.tile([97, NL * O], BF16, tag="wt")
                        wt_i8 = wt_bf[0:96, :].rearrange(
                            "p (nh o nl) -> p nh o nl", o=O, nl=8)
                        wt_g = wt_bf[:].rearrange(
                            "p (nh o nl) -> p nh nl o", o=O, nl=8)
                        nc.gpsimd.dma_start(wt_bf[96:97, :], bias_flat[:, :])

                        for ch in range(NT):
                            nc.gpsimd.dma_start(
                                xgs[ch][C + 16:2 * C, :].rearrange(
                                    "c (n b) -> c n b", b=B),
                                scr1[16:32, ch * 128:(ch + 1) * 128, :],
                            )

                        # ---- weight-slab generation (PE hot off sweep B) ---
                        with tc.tile_pool(name="ps_wt", bufs=4, space="PSUM") as ps_wt:
                            for o in range(O):
                                w_ps = ps_wt.tile([96, NL], F32, tag="wps")
                                nc.tensor.matmul(
                                    w_ps[:],
                                    wpo_sb[:, o * (CHEB_K * C):(o + 1) * (CHEB_K * C)],
                                    etl_bf[:],
                                    start=True, stop=True,
                                )
                                src_v = w_ps[:].rearrange(
                                    "p (nh nl) -> p nh nl", nl=8)
                                if o % 2 == 0:
                                    nc.vector.tensor_copy(wt_i8[:, :, o, :], src_v)
                                else:
                                    nc.scalar.activation(
                                        wt_i8[:, :, o, :], src_v, AF.Copy)

                        # ---- hop2: h-major, k-inner; rhs from ag_out -------
                        with tc.tile_pool(name="ps_h2", bufs=1, space="PSUM") as ps_h2:
                            acc2 = [
                                ps_h2.tile([128, 512], F32, tag=f"a2_{nt_i}_{h}",
                                           name=f"a2_{nt_i}_{h}")
                                for nt_i in range(NT) for h in range(2)
                            ]

                            def drain2(h):
                                wr_engs = [nc.gpsimd, nc.sync, nc.gpsimd, nc.sync]
                                for nt_i in range(NT):
                                    a = acc2[nt_i * 2 + h]
                                    st = tstream.tile([128, 512], F32, tag="zst")
                                    nc.scalar.activation(
                                        st[:], a[:], AF.Copy,
                                        scale=r2[:, nt_i:nt_i + 1],
                                    )
                                    st_bf = tstream.tile([128, 512], BF16, tag="zstb")
                                    nc.vector.tensor_tensor(
                                        st_bf[:], st[:],
                                        xloc_sb[:, nt_i * BC + h * 512:
                                                nt_i * BC + (h + 1) * 512],
                                        mybir.AluOpType.subtract,
                                    )
                                    nc.gpsimd.dma_start(
                                        scr2[h * 16:(h + 1) * 16,
                                             nt_i * 128:(nt_i + 1) * 128, :]
                                        .transpose((1, 0, 2)),
                                        st_bf[:].rearrange("p (c b) -> p c b", b=B),
                                    )
                                    # xg z2 rows for this chunk, right after
                                    # its scr2 slice lands (h-half rows)
                                    wr_engs[nt_i].dma_start(
                                        xgs[nt_i][2 * C + 16 * h:
                                                  2 * C + 16 * (h + 1), :]
                                        .rearrange("c (n b) -> c n b", b=B),
                                        scr2[16 * h:16 * (h + 1),
                                             nt_i * 128:(nt_i + 1) * 128, :],
                                    )

                            for h in range(2):
                                for k in range(MT):
                                    rt = stream.tile([128, 512], BF16, tag="rhs2")
                                    nc.sync.dma_start(
                                        rt[:], ag_outs[h][k * 128:(k + 1) * 128, :])
                                    for nt_i in range(NT):
                                        nc.tensor.matmul(
                                            acc2[nt_i * 2 + h][:],
                                            pt[:, k * NL + nt_i * 128:
                                               k * NL + (nt_i + 1) * 128],
                                            rt[:, :],
                                            start=(k == 0), stop=(k == MT - 1),
                                        )
                                drain2(h)

                        # ---- grouped per-node GEMM (col-tiled via tile_position)
                        with tc.tile_pool(name="ps_g", bufs=4, space="PSUM") as ps_g:
                            dma_engs = [nc.sync, nc.gpsimd, nc.scalar]
                            for ch in range(NT):  # 128 nodes per chunk
                                n0 = ch * 128
                                xg_b = xgs[ch]
                                for q16 in range(4):  # 32 nodes per psum tile
                                    g_ps = ps_g.tile([128, 512], F32, tag="gps")
                                    for j in range(8):
                                        for g in range(4):
                                            nl_i = q16 * 32 + j * 4 + g
                                            n_gl = n0 + nl_i
                                            nc.tensor.matmul(
                                                g_ps[32 * g:32 * (g + 1),
                                                     j * O:(j + 1) * O],
                                                xg_b[:, nl_i * B:(nl_i + 1) * B],
                                                wt_g[:, n_gl // 8, n_gl % 8, :],
                                                start=True, stop=True,
                                                tile_position=(0, 32 * g),
                                            )
                                    st = tstream.tile([128, 512], F32, tag="gst")
                                    idx = ch * 4 + q16
                                    if idx % 2 == 0:
                                        nc.vector.tensor_copy(st[:], g_ps[:])
                                    else:
                                        nc.scalar.activation(st[:], g_ps[:], AF.Copy)
                                    dma_engs[idx % 3].dma_start(
                                        out_hw[idx * 128:(idx + 1) * 128, :], st[:])
    return out_hw


_COMPILED = None


def _get_compiled():
    global _COMPILED
    if _COMPILED is None:
        nc = bacc.Bacc(
            "TRN2",
            target_bir_lowering=False,
            debug=False,
            num_devices=NC_CORES,
        )
        _build(nc)
        nc.compile()
        _COMPILED = nc
    return _COMPILED


def kernel(x, node_embeddings, laplacian_mx, weights_pool, bias_pool):
    x = np.asarray(x, dtype=np.float32)
    e = np.asarray(node_embeddings, dtype=np.float32)
    wp = np.asarray(weights_pool, dtype=np.float32)
    bp = np.asarray(bias_pool, dtype=np.float32)

    et = np.ascontiguousarray(e.T)                                  # [D, N]
    xt_h = np.ascontiguousarray(x.transpose(1, 2, 0).reshape(N, BC))  # [m, c*32+b]
    wpo = np.ascontiguousarray(wp.transpose(3, 0, 1, 2).reshape(O, D, CHEB_K * C)).astype(ml_dtypes.bfloat16)
    bias_h = (e @ bp).astype(np.float32)                            # [N, O]

    xt_b = xt_h.astype(ml_dtypes.bfloat16)
    ones_row = np.ones((1, 128 * B), dtype=ml_dtypes.bfloat16)
    in_maps = []
    for i in range(NC_CORES):
        sl = slice(i * NL, (i + 1) * NL)
        in_maps.append({
            "et": et,
            "et_loc": np.ascontiguousarray(et[:, sl]),
            "xt_bf": xt_b,
            "xtl_bf": np.ascontiguousarray(xt_b[sl]),
            "xt_loc": np.ascontiguousarray(xt_h[sl]),
            "wp_o": wpo,
            "ones_row": ones_row,
            "bias_flat": np.ascontiguousarray(
                bias_h[sl].reshape(64, 8, O).transpose(0, 2, 1).reshape(1, NL * O).astype(ml_dtypes.bfloat16)
            ),
        })

    nc = _get_compiled()
    trace = bool(int(os.environ.get("KBENCH_TRACE", "0")))
    if trace:
        trace = _register_ntff_hook()
    res = run_bass_kernel_spmd(
        nc,
        in_maps,
        core_ids=list(range(NC_CORES)),
        trace=trace,
    )
    LAST_RESULTS["exec_time_ns"] = res.exec_time_ns
    LAST_RESULTS["trace"] = res.instructions_and_trace
    LAST_RESULTS["mean_exec_time_ns"] = res.mean_exec_time_ns

    out = np.empty((B, N, O), dtype=np.float32)
    for i in range(NC_CORES):
        raw = res.results[i]["out_hw"]                    # [2048, 512]
        o6 = raw.reshape(4, 4, 4, 32, 8, 64)              # [ch, q16, g, b, j, o]
        out[:, i * NL:(i + 1) * NL, :] = (
            o6.transpose(3, 0, 1, 4, 2, 5).reshape(B, NL, O))
    return out


# revision 11
# speedup vs baseline: 1.0207x; 1.0207x over previous
"""Trainium2 Bass kernel for nn_Embedded_GCN (gnn_message_passing).

Reference math (B=32, N=4096, C=32, O=64, D=16, K=3):
  A  = softmax(relu(E @ E.T), axis=1)              # [N, N] adaptive adjacency
  T0 = I, T1 = A, T2 = 2A@A - I                    # Chebyshev
  x_g[k]   = T_k @ x_b  for each batch             # [B, K, N, C]
  W[n]     = sum_d E[n,d] * Wp[d]                  # per-node weights [K,C,O]
  out[b,n] = sum_{k,i} x_g[b,n,k,i] W[n,k,i,:] + E[n]@bias_pool

Schedule (v4): hops split into bc-column halves; scores+exp software-pipelined
into hop1 sweep A (PE never idles behind the Act exp chain); xt's second
column half is cached in SBUF during the fused phase so sweep B needs no DMA
while AllGather #1 is in flight; a tiny warmup AllGather at program start
absorbs the inter-core skew so AG1's mesh handshake is short; hop2 consumes
ag_out0 for h=0 right after weight-gen and ag_out1 for h=1. Grouped-GEMM
output is dumped in raw PSUM-tile layout (contiguous) and reassembled on the
host.

  * z1 = A@x, z2 = 2*A@z1 - x (T2 never materialized).
  * softmax(relu(s)) = max(1, exp(s)) / rowsum.
  * PT[m, n] = transposed exp-scores (contraction m on partitions).
  * Scores f32r; both hops bf16.
  * Row sums: DVE/Pool split interleaved accumulation of exp tiles, then 4
    tiny PE ones-matmuls; 1/Z folded into the hop epilogues.
  * Per-node grouped GEMM: x_g permuted to [(k,c), (n,b)] via a DRAM
    round-trip, per-node weights generated on the PE into a
    [97, (n_hi, o, n_lo=8)]-interleaved bf16 slab (97th row = bias), and the
    512 per-node [97,32]x[97,64] bf16 matmuls run col-tiled via tile_position.
"""

import os

import numpy as np
import ml_dtypes

import concourse.bass as bass
import concourse.mybir as mybir
import concourse.tile as tile
from concourse import bacc
from concourse.bass_utils import run_bass_kernel_spmd

F32 = mybir.dt.float32
F32R = mybir.dt.float32r
BF16 = mybir.dt.bfloat16
AF = mybir.ActivationFunctionType

B, N, C, O, D, CHEB_K = 32, 4096, 32, 64, 16, 3
NC_CORES = 8
NL = N // NC_CORES          # 512 nodes per core
BC = B * C                  # 1024
MT = N // 128               # 32 contraction tiles
NT = NL // 128              # 4 local node tiles

LAST_RESULTS = {}


def _register_ntff_hook():
    """Inject antenv.axon_hooks (absent from the container's antenv stub) and
    register the ctypes NTFF-profile hook so trace=True works under axon."""
    import sys
    import types

    try:
        import antenv

        if "antenv.axon_hooks" not in sys.modules:
            mod = types.ModuleType("antenv.axon_hooks")
            mod._hook = None

            def set_axon_ntff_profile_hook(h):
                mod._hook = h

            def get_axon_ntff_profile_hook():
                return mod._hook

            mod.set_axon_ntff_profile_hook = set_axon_ntff_profile_hook
            mod.get_axon_ntff_profile_hook = get_axon_ntff_profile_hook
            sys.modules["antenv.axon_hooks"] = mod
            antenv.axon_hooks = mod

        hooks = sys.modules["antenv.axon_hooks"]
        if hooks.get_axon_ntff_profile_hook() is None:
            from trn_agent_boot.trn_boot import _ntff_profile_via_ctypes

            hook = _ntff_profile_via_ctypes("/opt/axon/libaxon_pjrt.so")
            if hook is not None:
                hooks.set_axon_ntff_profile_hook(hook)
        return True
    except Exception:
        return False


def _build(nc: bacc.Bacc):
    # ---- I/O -------------------------------------------------------------
    et = nc.dram_tensor("et", [D, N], F32, kind="ExternalInput")          # E^T
    et_loc = nc.dram_tensor("et_loc", [D, NL], F32, kind="ExternalInput")
    xt_bf = nc.dram_tensor("xt_bf", [N, BC], BF16, kind="ExternalInput")   # x[b,m,c] -> [m, c*32+b]
    xtl_bf = nc.dram_tensor("xtl_bf", [NL, BC], BF16, kind="ExternalInput")
    xt_loc = nc.dram_tensor("xt_loc", [NL, BC], F32, kind="ExternalInput")
    wp_o = nc.dram_tensor("wp_o", [O, D, CHEB_K * C], BF16, kind="ExternalInput")
    bias_flat = nc.dram_tensor("bias_flat", [1, NL * O], BF16, kind="ExternalInput")
    ones_row = nc.dram_tensor("ones_row", [1, 128 * B], BF16, kind="ExternalInput")
    # raw grouped-GEMM PSUM dump: [(ch,q16)*128 + (g,b), (j,o)]; host reassembles
    out_hw = nc.dram_tensor("out_hw", [16 * 128, 512], F32, kind="ExternalOutput")

    with tile.TileContext(nc) as tc:
        with tc.tile_pool(name="dram", bufs=1, space="DRAM") as dram, \
             tc.tile_pool(name="persist", bufs=1) as persist:

            # z1 column halves: ag_in[q] = local 512 nodes x bc half q
            ag_ins = [dram.tile([NL, 512], BF16, tag=f"ag_in{q}", name=f"ag_in{q}")
                      for q in range(2)]
            ag_outs = [dram.tile([N, 512], BF16, tag=f"ag_out{q}",
                                 name=f"ag_out{q}", addr_space="Shared")
                       for q in range(2)]
            scr1 = dram.tile([C, NL, B], BF16, tag="scr1")   # z1 as [c, n, b]
            scr2 = dram.tile([C, NL, B], BF16, tag="scr2")   # z2 as [c, n, b]
            # ---- small persistent SBUF ------------------------------------
            etl_sb = persist.tile([D, NL], F32R, tag="etl")
            r1 = persist.tile([128, NT], F32, tag="r1")          # 1/Z  per node col nt
            r2 = persist.tile([128, NT], F32, tag="r2")          # 2/Z
            ones_f = persist.tile([128, 2], F32, tag="onesf")
            etl_bf = persist.tile([D, NL], BF16, tag="etlbf")
            wpo_sb = persist.tile([D, O * CHEB_K * C], BF16, tag="wpo")

            nc.sync.dma_start(etl_sb[:], et_loc[:, :].bitcast(F32R))
            nc.vector.memset(ones_f[:], 1.0)
            nc.vector.tensor_copy(etl_bf[:], etl_sb[:].bitcast(F32))

            with tc.tile_pool(name="xgp", bufs=1) as xgp, \
                 tc.tile_pool(name="tstream", bufs=3) as tstream:
                xgs = [xgp.tile([97, 128 * B], BF16, tag=f"xg{ch}", name=f"xg{ch}")
                       for ch in range(NT)]

                with tc.tile_pool(name="ptp", bufs=1) as ptp, \
                     tc.tile_pool(name="stream", bufs=4) as stream:
                    pt = ptp.tile([128, MT * NL], BF16, tag="pt")  # PT[m%128, mt*NL+n]
                    xloc_sb = ptp.tile([128, NT * BC], F32, tag="xloc")
                    accs = [ptp.tile([128, NL], F32, tag=f"accs{i}", name=f"accs{i}")
                            for i in range(2)]

                    # ==== early epoch: fused scores+sweepA, sweepB (SBUF) ====
                    xt2p_cm = tc.tile_pool(name="xt2p", bufs=1)
                    xt2p = xt2p_cm.__enter__()
                    xt2 = xt2p.tile([128, MT * 512], BF16, tag="xt2")
                    z1sb = [None] * (2 * NT)

                    ps_acc_cm = tc.tile_pool(name="ps_acc", bufs=1, space="PSUM")
                    ps_acc = ps_acc_cm.__enter__()
                    acc = [ps_acc.tile([128, 512], F32, tag=f"acc{nt_i}",
                                       name=f"acc{nt_i}") for nt_i in range(NT)]

                    def hop1_mms(k, rhs_ap):
                        for nt_i in range(NT):
                            nc.tensor.matmul(
                                acc[nt_i][:],
                                pt[:, k * NL + nt_i * 128: k * NL + (nt_i + 1) * 128],
                                rhs_ap,
                                start=(k == 0), stop=(k == MT - 1),
                            )

                    with tc.tile_pool(name="etp", bufs=2) as etp, \
                         tc.tile_pool(name="ps_sc", bufs=2, space="PSUM") as ps_sc, \
                         tc.tile_pool(name="ps_zs", bufs=1, space="PSUM") as ps_zs:
                        et_c = None
                        rts = [None] * MT
                        for mt in range(MT):
                            if mt % 8 == 0:
                                et_c = etp.tile([D, 1024], F32R, tag="etc")
                                nc.sync.dma_start(
                                    et_c[:],
                                    et[:, mt * 128:(mt + 8) * 128].bitcast(F32R))
                            s_ps = ps_sc.tile([128, NL], F32, tag="s")
                            nc.tensor.matmul(
                                s_ps[:],
                                et_c[:, (mt % 8) * 128:(mt % 8 + 1) * 128],
                                etl_sb[:],
                                start=True, stop=True,
                            )
                            pslice = pt[:, mt * NL:(mt + 1) * NL]
                            nc.scalar.activation(pslice, s_ps[:], AF.Exp)
                            nc.vector.tensor_scalar_max(pslice, pslice, 1.0)
                            # rowsum accumulation split across DVE and Pool
                            eng = nc.vector if mt % 2 == 0 else nc.gpsimd
                            a = accs[mt % 2]
                            if mt < 2:
                                eng.tensor_copy(a[:], pslice)
                            else:
                                eng.tensor_tensor(
                                    a[:], a[:], pslice, mybir.AluOpType.add)
                            rt = stream.tile([128, 512], BF16, tag="rhs")
                            nc.sync.dma_start(
                                rt[:], xt_bf[mt * 128:(mt + 1) * 128, 0:512])
                            rts[mt] = rt
                            # xt second-half cache fill (for DMA-free sweep B)
                            nc.scalar.dma_start(
                                xt2[:, mt * 512:(mt + 1) * 512],
                                xt_bf[mt * 128:(mt + 1) * 128, 512:1024])
                            if mt == 0:
                                nc.scalar.dma_start(
                                    wpo_sb[:].rearrange(
                                        "d (o k) -> d o k", k=CHEB_K * C),
                                    wp_o[:, :, :].transpose((1, 0, 2)),
                                )
                            if 4 <= mt < 8:
                                ch = mt - 4
                                nc.gpsimd.dma_start(
                                    xgs[ch][0:C, :].rearrange("c (n b) -> c n b", b=B),
                                    xtl_bf[ch * 128:(ch + 1) * 128, :].rearrange(
                                        "n (c b) -> c n b", b=B),
                                )
                            if 10 <= mt < 14:
                                nc.gpsimd.dma_start(
                                    xgs[mt - 10][96:97, :], ones_row[:, :])
                            if mt == 16:
                                nc.gpsimd.dma_start(
                                    xloc_sb[:].rearrange("p (t f) -> p t f", f=BC),
                                    xt_loc[:, :].rearrange("(t p) f -> p t f", p=128),
                                )
                            if mt >= 2:
                                hop1_mms(mt - 2, rts[mt - 2][:, :])
                                rts[mt - 2] = None
                        hop1_mms(MT - 2, rts[MT - 2][:, :])
                        hop1_mms(MT - 1, rts[MT - 1][:, :])

                        # row sums -> r1 = 1/Z, r2 = 2/Z
                        nc.vector.tensor_tensor(
                            accs[0][:], accs[0][:], accs[1][:], mybir.AluOpType.add)
                        zs = ps_zs.tile([128, 8], F32, tag="zs")
                        for nt_i in range(NT):
                            nc.tensor.matmul(
                                zs[:, 2 * nt_i:2 * nt_i + 2],
                                accs[0][:, nt_i * 128:(nt_i + 1) * 128],
                                ones_f[:],
                                start=True, stop=True,
                            )
                        for nt_i in range(NT):
                            nc.vector.reciprocal(
                                r1[:, nt_i:nt_i + 1], zs[:, 2 * nt_i:2 * nt_i + 1])
                            nc.vector.tensor_scalar_mul(
                                r2[:, nt_i:nt_i + 1], r1[:, nt_i:nt_i + 1], 2.0)

                    # ---- drain sweep A (h=0) -> AG1; sweep B (h=1) -> AG2 -
                    def drain(h):
                        for nt_i in range(NT):
                            st_bf = xt2p.tile([128, 512], BF16, tag=f"zb{h}_{nt_i}",
                                              name=f"zb{h}_{nt_i}")
                            nc.vector.tensor_scalar(
                                st_bf[:], acc[nt_i][:], r1[:, nt_i:nt_i + 1], None,
                                op0=mybir.AluOpType.mult,
                            )
                            z1sb[h * NT + nt_i] = st_bf
                            nc.sync.dma_start(
                                ag_ins[h][nt_i * 128:(nt_i + 1) * 128, :], st_bf[:])
                            nc.scalar.dma_start(
                                scr1[h * 16:(h + 1) * 16,
                                     nt_i * 128:(nt_i + 1) * 128, :]
                                .transpose((1, 0, 2)),
                                st_bf[:].rearrange("p (c b) -> p c b", b=B),
                            )
                        nc.gpsimd.collective_compute(
                            "AllGather",
                            mybir.AluOpType.bypass,
                            ins=[ag_ins[h].opt()],
                            outs=[ag_outs[h].opt()],
                            replica_groups=[list(range(NC_CORES))],
                        )
                        for ch in range(NT):
                            nc.gpsimd.dma_start(
                                xgs[ch][C + 16 * h:C + 16 * (h + 1), :]
                                .rearrange("c (n b) -> c n b", b=B),
                                scr1[16 * h:16 * (h + 1),
                                     ch * 128:(ch + 1) * 128, :],
                            )

                    drain(0)

                    # sweep B: same PT, bc cols 512:1024 from SBUF cache
                    for k in range(MT):
                        hop1_mms(k, xt2[:, k * 512:(k + 1) * 512])

                    drain(1)
                    ps_acc_cm.__exit__(None, None, None)
                    xt2p_cm.__exit__(None, None, None)

                    # ==== late epoch: weight slab + hop2 + grouped ====
                    with tc.tile_pool(name="wtp", bufs=1) as wtp:
                        # weight slab, n-major: [ki|bias, (n, o)]
                        wt_bf = wtp.tile([97, NL * O], BF16, tag="wt")
                        wt_i8 = wt_bf[0:96, :].rearrange(
                            "p (nh o nl) -> p nh o nl", o=O, nl=8)
                        wt_g = wt_bf[:].rearrange(
                            "p (nh o nl) -> p nh nl o", o=O, nl=8)
                        nc.gpsimd.dma_start(wt_bf[96:97, :], bias_flat[:, :])

                        # ---- weight-slab generation (PE hot off sweep B) ---
                        with tc.tile_pool(name="ps_wt", bufs=4, space="PSUM") as ps_wt:
                            for o in range(O):
                                w_ps = ps_wt.tile([96, NL], F32, tag="wps")
                                nc.tensor.matmul(
                                    w_ps[:],
                                    wpo_sb[:, o * (CHEB_K * C):(o + 1) * (CHEB_K * C)],
                                    etl_bf[:],
                                    start=True, stop=True,
                                )
                                src_v = w_ps[:].rearrange(
                                    "p (nh nl) -> p nh nl", nl=8)
                                if o % 2 == 0:
                                    nc.vector.tensor_copy(wt_i8[:, :, o, :], src_v)
                                else:
                                    nc.scalar.activation(
                                        wt_i8[:, :, o, :], src_v, AF.Copy)

                        # ---- hop2: h-major, k-inner; rhs from ag_out -------
                        with tc.tile_pool(name="ps_h2", bufs=1, space="PSUM") as ps_h2:
                            acc2 = [
                                ps_h2.tile([128, 512], F32, tag=f"a2_{nt_i}_{h}",
                                           name=f"a2_{nt_i}_{h}")
                                for nt_i in range(NT) for h in range(2)
                            ]

                            def drain2(h):
                                wr_engs = [nc.gpsimd, nc.scalar, nc.gpsimd, nc.scalar]
                                for nt_i in range(NT):
                                    a = acc2[nt_i * 2 + h]
                                    st = tstream.tile([128, 512], F32, tag="zst")
                                    nc.scalar.activation(
                                        st[:], a[:], AF.Copy,
                                        scale=r2[:, nt_i:nt_i + 1],
                                    )
                                    st_bf = tstream.tile([128, 512], BF16, tag="zstb")
                                    nc.vector.tensor_tensor(
                                        st_bf[:], st[:],
                                        xloc_sb[:, nt_i * BC + h * 512:
                                                nt_i * BC + (h + 1) * 512],
                                        mybir.AluOpType.subtract,
                                    )
                                    wr_engs[nt_i].dma_start(
                                        scr2[h * 16:(h + 1) * 16,
                                             nt_i * 128:(nt_i + 1) * 128, :]
                                        .transpose((1, 0, 2)),
                                        st_bf[:].rearrange("p (c b) -> p c b", b=B),
                                    )
                                    wr_engs[nt_i].dma_start(
                                        xgs[nt_i][2 * C + 16 * h:
                                                  2 * C + 16 * (h + 1), :]
                                        .rearrange("c (n b) -> c n b", b=B),
                                        scr2[16 * h:16 * (h + 1),
                                             nt_i * 128:(nt_i + 1) * 128, :],
                                    )

                            for h in range(2):
                                for k in range(MT):
                                    rt = stream.tile([128, 512], BF16, tag="rhs2")
                                    nc.sync.dma_start(
                                        rt[:], ag_outs[h][k * 128:(k + 1) * 128, :])
                                    for nt_i in range(NT):
                                        nc.tensor.matmul(
                                            acc2[nt_i * 2 + h][:],
                                            pt[:, k * NL + nt_i * 128:
                                               k * NL + (nt_i + 1) * 128],
                                            rt[:, :],
                                            start=(k == 0), stop=(k == MT - 1),
                                        )
                                drain2(h)

                        # ---- grouped per-node GEMM (col-tiled via tile_position)
                        with tc.tile_pool(name="ps_g", bufs=4, space="PSUM") as ps_g:
                            dma_engs = [nc.sync, nc.gpsimd, nc.scalar]
                            for ch in range(NT):  # 128 nodes per chunk
                                n0 = ch * 128
                                xg_b = xgs[ch]
                                for q16 in range(4):  # 32 nodes per psum tile
                                    g_ps = ps_g.tile([128, 512], F32, tag="gps")
                                    for j in range(8):
                                        for g in range(4):
                                            nl_i = q16 * 32 + j * 4 + g
                                            n_gl = n0 + nl_i
                                            nc.tensor.matmul(
                                                g_ps[32 * g:32 * (g + 1),
                                                     j * O:(j + 1) * O],
                                                xg_b[:, nl_i * B:(nl_i + 1) * B],
                                                wt_g[:, n_gl // 8, n_gl % 8, :],
                                                start=True, stop=True,
                                                tile_position=(0, 32 * g),
                                            )
                                    st = tstream.tile([128, 512], F32, tag="gst")
                                    idx = ch * 4 + q16
                                    if idx % 2 == 0:
                                        nc.vector.tensor_copy(st[:], g_ps[:])
                                    else:
                                        nc.scalar.activation(st[:], g_ps[:], AF.Copy)
                                    dma_engs[idx % 3].dma_start(
                                        out_hw[idx * 128:(idx + 1) * 128, :], st[:])
    return out_hw


_COMPILED = None


def _get_compiled():
    global _COMPILED
    if _COMPILED is None:
        nc = bacc.Bacc(
            "TRN2",
            target_bir_lowering=False,
            debug=False,
            num_devices=NC_CORES,
        )
        _build(nc)
        nc.compile()
        _COMPILED = nc
    return _COMPILED


def kernel(x, node_embeddings, laplacian_mx, weights_pool, bias_pool):
    x = np.asarray(x, dtype=np.float32)
    e = np.asarray(node_embeddings, dtype=np.float32)
    wp = np.asarray(weights_pool, dtype=np.float32)
    bp = np.asarray(bias_pool, dtype=np.float32)

    et = np.ascontiguousarray(e.T)                                  # [D, N]
    xt_h = np.ascontiguousarray(x.transpose(1, 2, 0).reshape(N, BC))  # [m, c*32+b]
    wpo = np.ascontiguousarray(wp.transpose(3, 0, 1, 2).reshape(O, D, CHEB_K * C)).astype(ml_dtypes.bfloat16)
    bias_h = (e @ bp).astype(np.float32)                            # [N, O]

    xt_b = xt_h.astype(ml_dtypes.bfloat16)
    ones_row = np.ones((1, 128 * B), dtype=ml_dtypes.bfloat16)
    in_maps = []
    for i in range(NC_CORES):
        sl = slice(i * NL, (i + 1) * NL)
        in_maps.append({
            "et": et,
            "et_loc": np.ascontiguousarray(et[:, sl]),
            "xt_bf": xt_b,
            "xtl_bf": np.ascontiguousarray(xt_b[sl]),
            "xt_loc": np.ascontiguousarray(xt_h[sl]),
            "wp_o": wpo,
            "ones_row": ones_row,
            "bias_flat": np.ascontiguousarray(
                bias_h[sl].reshape(64, 8, O).transpose(0, 2, 1).reshape(1, NL * O).astype(ml_dtypes.bfloat16)
            ),
        })

    nc = _get_compiled()
    trace = bool(int(os.environ.get("KBENCH_TRACE", "0")))
    if trace:
        trace = _register_ntff_hook()
    res = run_bass_kernel_spmd(
        nc,
        in_maps,
        core_ids=list(range(NC_CORES)),
        trace=trace,
    )
    LAST_RESULTS["exec_time_ns"] = res.exec_time_ns
    LAST_RESULTS["trace"] = res.instructions_and_trace
    LAST_RESULTS["mean_exec_time_ns"] = res.mean_exec_time_ns

    out = np.empty((B, N, O), dtype=np.float32)
    for i in range(NC_CORES):
        raw = res.results[i]["out_hw"]                    # [2048, 512]
        o6 = raw.reshape(4, 4, 4, 32, 8, 64)              # [ch, q16, g, b, j, o]
        out[:, i * NL:(i + 1) * NL, :] = (
            o6.transpose(3, 0, 1, 4, 2, 5).reshape(B, NL, O))
    return out
